# revision 1
# baseline (speedup 1.0000x reference)
"""TRN2 Bass kernel for nn_DecayModel: bidirectional decay scan (d=0.5).

Math: out[i] = (fwd[i] + bwd[i]) / norm[i] where
  fwd[i] = sum_{k<=i} d^{i-k} x[k],  bwd[i] = sum_{k>=i} d^{k-i} x[k]
  => fwd + bwd = sum_k d^{|i-k|} x[k] + x[i]
  norm[i] = (2 - d^i) + (2 - d^{S-1-i}) = 4 - d^i - d^{S-1-i}

Since d = 0.5, d^j = 2^-j decays below fp32 significance within ~48 steps, so
the scan is a banded (Toeplitz) convolution along S. We compute it as matmuls
over 128-row S-tiles: out_tile[t] = Wc@x[t] + Wp@x[t-1] + Wn@x[t+1], with the
three weight matrices made of exact powers of two, accumulated in PSUM, then
scaled by 1/norm (per-partition scalar) on eviction.

Sharding: data-parallel over batch. B=16 across 8 cores -> 2 batches/core,
flattened to [4096, 1024] (32 S-tiles; tiles 0-15 batch 0, 16-31 batch 1).
"""
import sys

sys.path.insert(0, "/opt/trn_rl_repo")

import numpy as np

import concourse.bass as bass
import concourse.tile as tile
from concourse import bacc, mybir
from concourse.bass_utils import run_bass_kernel_spmd

DECAY = 0.5
B, S, H = 16, 2048, 1024
N_CORES = 8
BPC = B // N_CORES          # batches per core
P = 128                     # S-tile rows (partitions)
TPB = S // P                # S-tiles per batch (16)
T = BPC * TPB               # S-tiles per core (32)
NCH = 512                   # matmul moving free-dim (fp32 max, 1 PSUM bank)
HCH = H // NCH              # H chunks per tile (2)


def _weights():
    """Constant numpy weights: Wc/Wp/Wn lhsT matrices + 1/norm table."""
    a = np.arange(P)
    # center: M_c[a,b] = d^|a-b| + delta(a,b); symmetric so lhsT == M_c
    wc = DECAY ** np.abs(a[:, None] - a[None, :]) + np.eye(P)
    # prev tile: M_p[a,b] = d^(P+a-b); lhsT_prev[b,a] = M_p[a,b]
    wp_lhsT = DECAY ** (P + a[None, :] - a[:, None])  # [b, a]
    # next tile: M_n[a,b] = d^(P+b-a); lhsT_next[b,a] = M_n[a,b] = wp_lhsT.T
    wn_lhsT = wp_lhsT.T.copy()
    # zero negligible entries (they'd be fp32 subnormals anyway)
    for w in (wc, wp_lhsT, wn_lhsT):
        w[w < 2.0**-60] = 0.0
    i = np.arange(S, dtype=np.float64)
    norm = 4.0 - DECAY**i - DECAY ** (S - 1.0 - i)
    rnorm = (1.0 / norm).astype(np.float32)  # [S]
    # [P, TPB]: column j = S-tile j within a batch, row a = position in tile
    rnorm_pt = rnorm.reshape(TPB, P).T.copy()
    return (
        wc.astype(np.float32),
        wp_lhsT.astype(np.float32),
        wn_lhsT.astype(np.float32),
        rnorm_pt,
    )


def _build(repeat=1, use_f32r=True, store_eng="sync", pair_dma=False):
    nc = bacc.Bacc("TRN2", target_bir_lowering=False, debug=False,
                   num_devices=N_CORES)
    x_d = nc.dram_tensor("x", [T * P, H], mybir.dt.float32, kind="ExternalInput")
    wc_d = nc.dram_tensor("wc", [P, P], mybir.dt.float32, kind="ExternalInput")
    wp_d = nc.dram_tensor("wp", [P, P], mybir.dt.float32, kind="ExternalInput")
    wn_d = nc.dram_tensor("wn", [P, P], mybir.dt.float32, kind="ExternalInput")
    rn_d = nc.dram_tensor("rnorm", [P, TPB], mybir.dt.float32, kind="ExternalInput")
    y_d = nc.dram_tensor("y", [T * P, H], mybir.dt.float32, kind="ExternalOutput")

    f32r = mybir.dt.float32r if use_f32r else mybir.dt.float32
    with tile.TileContext(nc) as tc:
        with (
            tc.tile_pool(name="const", bufs=1) as const_pool,
            tc.tile_pool(name="xp", bufs=6) as x_pool,
            tc.tile_pool(name="hp", bufs=12) as h_pool,
            tc.tile_pool(name="op", bufs=8) as out_pool,
            tc.tile_pool(name="ps", bufs=8, space="PSUM") as psum_pool,
        ):
            wc = const_pool.tile([P, P], f32r, tag="wc")
            wp = const_pool.tile([P, P], f32r, tag="wp")
            wn = const_pool.tile([P, P], f32r, tag="wn")
            rn = const_pool.tile([P, TPB], mybir.dt.float32, tag="rn")
            nc.sync.dma_start(wc[:], wc_d.ap()[:].bitcast(f32r))
            nc.sync.dma_start(wp[:], wp_d.ap()[:].bitcast(f32r))
            nc.sync.dma_start(wn[:], wn_d.ap()[:].bitcast(f32r))
            nc.sync.dma_start(rn[:], rn_d.ap()[:])

            hi_tiles = {}
            lo_tiles = {}

            def load(t, t2=None):
                # x -> (hi, lo) split: hi = f32r-rounded (12-bit) copy,
                # lo = exact fp32 residual (fits in 12 bits -> exact in f32r)
                if t2 is not None:
                    # paired 1MB load: rows of tiles t,t+1 -> one [P, 2, H] tile
                    xp2 = x_pool.tile([P, 2, H], mybir.dt.float32, tag="x")
                    nc.sync.dma_start(
                        xp2[:],
                        x_d.ap()[t * P:(t + 2) * P, :].rearrange(
                            "(two p) h -> p two h", two=2),
                    )
                    for i, tt_ in enumerate((t, t2)):
                        _split(tt_, xp2[:, i, :])
                    return
                xt = x_pool.tile([P, H], mybir.dt.float32, tag="x")
                nc.sync.dma_start(xt[:], x_d.ap()[t * P:(t + 1) * P, :])
                _split(t, xt)

            def _split(t, xt):
                src = xt if isinstance(xt, bass.AP) else xt[:]
                if not use_f32r:
                    # fp32 fallback: matmuls consume x directly, no split
                    hi_tiles[t] = src
                    lo_tiles[t] = None
                    return
                xh = h_pool.tile([P, H], f32r, tag="xh")
                nc.scalar.copy(xh[:], src)
                xl = h_pool.tile([P, H], f32r, tag="xl")
                nc.vector.tensor_sub(xl[:], src, xh[:].bitcast(mybir.dt.float32))
                hi_tiles[t] = xh
                lo_tiles[t] = xl

            def compute(t):
                tt = t % TPB  # S-tile index within its batch
                has_prev = tt != 0
                has_next = tt != TPB - 1
                ot = out_pool.tile([P, H], mybir.dt.float32, tag="o")
                for h in range(HCH):
                    sl = slice(h * NCH, (h + 1) * NCH)
                    pt = psum_pool.tile([P, NCH], mybir.dt.float32, tag="p")
                    # hi sweep then lo sweep: no same-weight-adjacent f32r
                    # matmuls (defensive: f32r weight reload quirks)
                    mms = [(wc, hi_tiles[t])]
                    if has_prev:
                        mms.append((wp, hi_tiles[t - 1]))
                    if has_next:
                        mms.append((wn, hi_tiles[t + 1]))
                    if use_f32r:
                        mms.append((wc, lo_tiles[t]))
                        if has_prev:
                            mms.append((wp, lo_tiles[t - 1]))
                        if has_next:
                            mms.append((wn, lo_tiles[t + 1]))
                    for i, (w, xt_) in enumerate(mms):
                        nc.tensor.matmul(pt[:], w[:], xt_[:, sl],
                                         start=(i == 0), stop=(i == len(mms) - 1))
                    # evict + normalize; alternate engines to balance load
                    if h == 0:
                        nc.vector.tensor_scalar_mul(ot[:, sl], pt[:],
                                                    rn[:, tt:tt + 1])
                    else:
                        nc.scalar.mul(ot[:, sl], pt[:], rn[:, tt:tt + 1])
                s_eng = nc.sync if store_eng == "sync" else nc.scalar
                s_eng.dma_start(y_d.ap()[t * P:(t + 1) * P, :], ot[:])

            for _ in range(repeat):
                hi_tiles.clear()
                lo_tiles.clear()
                if pair_dma:
                    load(0, 1)
                    for t in range(T):
                        if t % 2 == 0 and t + 2 < T:
                            load(t + 2, t + 3)
                        compute(t)
                else:
                    load(0)
                    load(1)
                    for t in range(T):
                        if t + 2 < T:
                            load(t + 2)
                        compute(t)

    nc.compile()
    return nc


_NC = None


def _get_nc():
    global _NC
    if _NC is None:
        _NC = _build()
    return _NC


def _in_maps(batch):
    wc, wp, wn, rn = _weights()
    maps = []
    for c in range(N_CORES):
        shard = np.ascontiguousarray(
            batch[c * BPC:(c + 1) * BPC].reshape(T * P, H), dtype=np.float32
        )
        maps.append({"x": shard, "wc": wc, "wp": wp, "wn": wn, "rnorm": rn})
    return maps


def kernel(batch, _trace=False):
    batch = np.asarray(batch, dtype=np.float32)
    assert batch.shape == (B, S, H), batch.shape
    maps = _in_maps(batch)
    res = None
    last_err = None
    # attempt 0-1: fast f32r-split kernel; attempt 2: plain-fp32 fallback
    for attempt in range(3):
        try:
            if attempt < 2:
                nc = _get_nc()
            else:
                nc = _build(use_f32r=False)
            res = run_bass_kernel_spmd(nc, maps, list(range(N_CORES)),
                                       trace=_trace)
            break
        except Exception as e:  # transient device wedge: retry
            last_err = e
            global _NC
            _NC = None
    if res is None:
        raise last_err
    out = np.empty((B, S, H), dtype=np.float32)
    for c in range(N_CORES):
        out[c * BPC:(c + 1) * BPC] = res.results[c]["y"].reshape(BPC, S, H)
    if _trace:
        return out, res
    return out



# revision 3
# speedup vs baseline: 25.6033x; 25.6033x over previous
"""TRN2 Bass kernel for nn_DecayModel: bidirectional decay scan (d=0.5).

Math: out[i] = (fwd[i] + bwd[i]) / norm[i] where
  fwd[i] = sum_{k<=i} d^{i-k} x[k],  bwd[i] = sum_{k>=i} d^{k-i} x[k]
  => fwd + bwd = sum_k d^{|i-k|} x[k] + x[i]
  norm[i] = 4 - d^i - d^{S-1-i}

d = 0.5 means d^j = 2^-j is insignificant beyond ~60 steps, so the scan is a
banded (Toeplitz) matmul along S. Key trick: load x on a 64-row-staggered
grid (window w = rows [128w-64, 128w+64) of the batch, zero-padded at the
ends) so the +/-60 band of every 128-row output tile is covered by exactly
TWO windows => 2 matmuls per PSUM chunk instead of 3 (prev/center/next).

Precision: correctness gate is rel-err < 2e-2; bf16 inputs/outputs give
~2.4e-3 (weights d^j and the +1 center tap are exact powers of two in bf16,
PSUM accumulates fp32). The fp32<->bf16 casts happen on the host, halving
HBM traffic (this is a memory-bound problem).

Sharding: data-parallel over batch. B=16 across 8 cores -> 2 batches/core.
Per-core x is [2*2176, H] bf16 (2176 = 64 pad + 2048 + 64 pad), y is
[4096, H] bf16. Loads are quad-packed ([128, 4, H] per DMA) on SyncE,
stores quad-packed on GpSimd. Evictions (x 1/norm, fp32 PSUM -> bf16)
alternate DVE/ACT.
"""
import sys

sys.path.insert(0, "/opt/trn_rl_repo")

import ml_dtypes
import numpy as np

import concourse.bass as bass
import concourse.tile as tile
from concourse import bacc, mybir
from concourse.bass_utils import run_bass_kernel_spmd

DECAY = 0.5
B, S, H = 16, 2048, 1024
N_CORES = 8
BPC = B // N_CORES          # batches per core (2)
P = 128                     # partition rows
TPB = S // P                # output S-tiles per batch (16)
NW = TPB + 1                # staggered windows per batch (17)
SPAD = S + P                # padded rows per batch (2176)
NCH = 512                   # matmul moving free-dim (1 PSUM bank fp32)
HCH = H // NCH              # H chunks per tile (2)
BAND = 60                   # d^j below 2^-60 is noise even for fp32
LGRP = 4                    # windows per load DMA
BF16 = ml_dtypes.bfloat16


def _weights():
    """WA/WB lhsT stationaries ([k, a] layout) + 1/norm table.

    Window A of output tile t holds batch rows [128t-64, 128t+64), window B
    holds [128t+64, 128t+192). Out row a, window row k:
      A: delta = a - k + 64,  B: delta = a - k - 64
    weight = d^|delta| (|delta| <= BAND) + delta(delta==0) (the extra x[i]
    from fwd+bwd double-counting word i). Every |delta|<=60 tap lands in
    exactly one of the two windows.
    """
    k = np.arange(P)[:, None]
    a = np.arange(P)[None, :]
    out = []
    for off in (64, -64):
        delta = a - k + off
        w = np.where(np.abs(delta) <= BAND, DECAY ** np.abs(delta), 0.0)
        w = w + (delta == 0)
        out.append(w.astype(BF16))
    i = np.arange(S, dtype=np.float64)
    norm = 4.0 - DECAY**i - DECAY ** (S - 1.0 - i)
    rnorm = (1.0 / norm).astype(np.float32)
    rnorm_pt = rnorm.reshape(TPB, P).T.copy()  # [P, TPB], col t = out tile t
    return out[0], out[1], rnorm_pt


def _build(repeat=1, wgrp=2, store_eng="gpsimd", load_eng="sync"):
    nc = bacc.Bacc("TRN2", target_bir_lowering=False, debug=False,
                   num_devices=N_CORES)
    bf16 = mybir.dt.bfloat16
    f32 = mybir.dt.float32
    x_d = nc.dram_tensor("x", [BPC * SPAD, H], bf16, kind="ExternalInput")
    wa_d = nc.dram_tensor("wa", [P, P], bf16, kind="ExternalInput")
    wb_d = nc.dram_tensor("wb", [P, P], bf16, kind="ExternalInput")
    rn_d = nc.dram_tensor("rnorm", [P, TPB], f32, kind="ExternalInput")
    y_d = nc.dram_tensor("y", [BPC * S, H], bf16, kind="ExternalOutput")

    l_eng = {"sync": nc.sync, "scalar": nc.scalar, "gpsimd": nc.gpsimd}[load_eng]
    s_eng = {"sync": nc.sync, "scalar": nc.scalar, "gpsimd": nc.gpsimd}[store_eng]

    with tile.TileContext(nc) as tc:
        with (
            tc.tile_pool(name="const", bufs=1) as const_pool,
            tc.tile_pool(name="xq", bufs=4) as xq_pool,
            tc.tile_pool(name="xs", bufs=2) as xs_pool,
            tc.tile_pool(name="oq", bufs=3) as oq_pool,
            tc.tile_pool(name="ps", bufs=8, space="PSUM") as ps_pool,
        ):
            wa = const_pool.tile([P, P], bf16, tag="wa")
            wb = const_pool.tile([P, P], bf16, tag="wb")
            rn = const_pool.tile([P, TPB], f32, tag="rn")
            nc.sync.dma_start(wa[:], wa_d.ap()[:])
            nc.sync.dma_start(wb[:], wb_d.ap()[:])
            nc.sync.dma_start(rn[:], rn_d.ap()[:])

            wins = {}       # (b, w) -> (tile, quad_idx or None)
            state = {"next_load": 0}
            # load sequence: per batch, 4 quads then the final single window
            seq = [(b, q) for b in range(BPC) for q in range(5)]

            def _issue_load(b, q):
                if q < 4:
                    t_ = xq_pool.tile([P, LGRP, H], bf16, tag="xq")
                    start = b * SPAD + P * LGRP * q
                    l_eng.dma_start(
                        t_[:],
                        x_d.ap()[start:start + P * LGRP, :].rearrange(
                            "(g p) h -> p g h", g=LGRP),
                    )
                    for i in range(LGRP):
                        wins[(b, LGRP * q + i)] = (t_, i)
                else:
                    t_ = xs_pool.tile([P, H], bf16, tag="xs")
                    start = b * SPAD + S
                    l_eng.dma_start(t_[:], x_d.ap()[start:start + P, :])
                    wins[(b, TPB)] = (t_, None)

            def ensure(n):
                while state["next_load"] <= min(n, len(seq) - 1):
                    _issue_load(*seq[state["next_load"]])
                    state["next_load"] += 1

            def win_ap(b, w, sl):
                t_, i = wins[(b, w)]
                return t_[:, i, sl] if i is not None else t_[:, sl]

            def compute_tile(b, t, oquad):
                # wgrp=1: A,B per chunk; wgrp=2: A,A,B,B across both chunks
                pts = [ps_pool.tile([P, NCH], f32, tag="p", name=f"p{c}")
                       for c in range(HCH)]
                sls = [slice(c * NCH, (c + 1) * NCH) for c in range(HCH)]
                if wgrp == 1:
                    for c in range(HCH):
                        nc.tensor.matmul(pts[c][:], wa[:], win_ap(b, t, sls[c]),
                                         start=True, stop=False)
                        nc.tensor.matmul(pts[c][:], wb[:], win_ap(b, t + 1, sls[c]),
                                         start=False, stop=True)
                else:
                    for c in range(HCH):
                        nc.tensor.matmul(pts[c][:], wa[:], win_ap(b, t, sls[c]),
                                         start=True, stop=False)
                    for c in range(HCH):
                        nc.tensor.matmul(pts[c][:], wb[:], win_ap(b, t + 1, sls[c]),
                                         start=False, stop=True)
                for c in range(HCH):
                    dst = oquad[:, t % 4, sls[c]]
                    if c % 2 == 0:
                        nc.vector.tensor_scalar_mul(dst, pts[c][:], rn[:, t:t + 1])
                    else:
                        nc.scalar.mul(dst, pts[c][:], rn[:, t:t + 1])

            for _ in range(repeat):
                wins.clear()
                state["next_load"] = 0
                ensure(1)
                for b in range(BPC):
                    for g in range(4):          # 4 output tiles per group
                        # group needs quads g, g+1 of batch b (+1 prefetch)
                        ensure(5 * b + g + 2)
                        oquad = oq_pool.tile([P, 4, H], bf16, tag="oq")
                        for t in range(4 * g, 4 * g + 4):
                            compute_tile(b, t, oquad)
                        start = b * S + 4 * P * g
                        s_eng.dma_start(
                            y_d.ap()[start:start + 4 * P, :].rearrange(
                                "(g p) h -> p g h", g=4),
                            oquad[:],
                        )

    nc.compile()
    return nc


_NC = None


def _get_nc():
    global _NC
    if _NC is None:
        _NC = _build()
    return _NC


def _in_maps(batch):
    wa, wb, rn = _weights()
    xb = np.asarray(batch, dtype=BF16).reshape(B, S, H)
    xpad = np.zeros((B, SPAD, H), dtype=BF16)
    xpad[:, P // 2:P // 2 + S] = xb
    maps = []
    for c in range(N_CORES):
        shard = np.ascontiguousarray(
            xpad[c * BPC:(c + 1) * BPC].reshape(BPC * SPAD, H))
        maps.append({"x": shard, "wa": wa, "wb": wb, "rnorm": rn})
    return maps


def kernel(batch, _trace=False):
    batch = np.asarray(batch, dtype=np.float32)
    assert batch.shape == (B, S, H), batch.shape
    maps = _in_maps(batch)
    res = None
    last_err = None
    for attempt in range(3):
        try:
            nc = _get_nc()
            res = run_bass_kernel_spmd(nc, maps, list(range(N_CORES)),
                                       trace=_trace)
            break
        except Exception as e:  # transient device wedge: retry
            last_err = e
            global _NC
            _NC = None
    if res is None:
        raise last_err
    out = np.empty((B, S, H), dtype=np.float32)
    for c in range(N_CORES):
        out[c * BPC:(c + 1) * BPC] = (
            res.results[c]["y"].astype(np.float32).reshape(BPC, S, H))
    if _trace:
        return out, res
    return out
